# revision 1
# baseline (speedup 1.0000x reference)
"""Multi-head attention (QKV Dense+LayerNorm projections, 16 heads, softmax
attention, output projection) for Trainium2, sharded over 8 NeuronCores.

Sharding: core c handles batch b = c//2, query-row half = c%2 (1024 query rows).
K/V projections for the batch are computed on both cores of the pair (the
LayerNorm couples the full feature dim, so K/V are duplicated instead of
communicated). Everything is fp32.

Per-core layout strategy:
  - Q/K are projected in "transposed" orientation: psum[feat_out, rows] =
    W_block.T @ X^T, so QK^T needs no on-chip transposes. LayerNorm stats
    (over the partition dim) come from ones-vector matmuls on TensorE, and
    the normalization x*A + C uses rank-1 A/C tiles built with K<=2 matmuls.
  - V is projected in natural orientation [rows, feat] (LayerNorm along the
    free dim), and stored as V' with a ones-column appended per head so the
    softmax denominator falls out of the P@V accumulation for free.
  - Attention per head-pair: simT[k_rows, q] = Kh @ Qh^T with 2 heads packed
    into the 128x128 PE array (row groups 0-63 / 64-127), exp on ScalarE
    (no max subtraction needed: LN-bounded logits), then OhT[dh+1, q] =
    V'^T-accumulation over k-blocks.
  - Output projection consumes the pair-stacked OhT tiles directly.
"""

import numpy as np

import concourse.bass as bass
import concourse.tile as tile
from concourse import bacc, mybir

FP = mybir.dt.float32
FR = mybir.dt.float32r
AF = mybir.ActivationFunctionType
OP = mybir.AluOpType

B, S, D, H, DH = 4, 2048, 1024, 16, 64
FI = D // 128          # 8 input-feature tiles
FO = D // 128          # 8 output-feature tiles
HP = H // 2            # 8 head pairs
EPS = 1e-5

N_CORES = 8
PROJ_STEPS = 4  # debug knob: 1=proj only, 2=+stats mms, 3=+stats chain, 4=full


def _proj_ln_transposed(nc, pools, xT, w_sb, gb, b_row, dst_d, rows):
    """Project + LayerNorm in transposed orientation; write [fo,128,rows] to
    dst_d (internal DRAM, [FO,128,rows]). xT: dram [128, FI, rows] input^T,
    w_sb: sbuf [128, FI, FO, 128] weight blocks, gb: sbuf [2, D] = (gain,bias)."""
    (xa_p, x2_p, stg_p, sm_p, ps_proj, ps_stats, ps_a, ps_c, consts) = pools
    ones_col = consts["ones_col"]
    for rc in range(rows // 512):
        x_all = xa_p.tile([128, FI, 512], FR, name="x_all", tag="x_all")
        nc.sync.dma_start(x_all[:], xT[:, :, rc * 512:(rc + 1) * 512])
        stats = ps_stats.tile([1, 512], FP, name="stats", tag="stats",
                              bufs=1)
        stats2 = ps_stats.tile([1, 512], FP, name="stats2", tag="stats2",
                               bufs=1)
        xs_sb = xa_p.tile([128, FO, 512], FR, name="xs_sb", tag="xs_sb")
        for fo in range(FO):
            pp = ps_proj.tile([128, 512], FP, name="pp", tag="pp")
            for fi in range(FI):
                nc.tensor.matmul(pp[:], w_sb[:, fi, fo, :], x_all[:, fi, :],
                                 start=(fi == 0), stop=(fi == FI - 1))
            x_sb = xs_sb[:, fo, :]
            nc.vector.tensor_copy(x_sb, pp[:])
            if PROJ_STEPS < 2:
                continue
            x2 = x2_p.tile([128, 512], FR, name="x2", tag="x2")
            nc.vector.tensor_mul(x2[:], x_sb, x_sb)
            nc.tensor.matmul(stats[0:1, :], ones_col[:], x_sb,
                             start=(fo == 0), stop=(fo == FO - 1))
            nc.tensor.matmul(stats2[0:1, :], ones_col[:], x2[:],
                             start=(fo == 0), stop=(fo == FO - 1))
        if PROJ_STEPS < 3:
            continue
        # stats -> rstd row, (-mu*rstd) row
        mu = sm_p.tile([1, 512], FP, name="mu", tag="mu")
        nc.vector.tensor_scalar(mu[:], stats[0:1, :], 1.0 / D, None, OP.mult)
        e2 = sm_p.tile([1, 512], FP, name="e2", tag="e2")
        nc.vector.tensor_scalar(e2[:], stats2[0:1, :], 1.0 / D, None,
                                OP.mult)
        var = sm_p.tile([1, 512], FP, name="var", tag="var")
        nc.vector.tensor_mul(var[:], mu[:], mu[:])
        # var = (e2 + eps) - mu^2
        nc.vector.scalar_tensor_tensor(var[:], e2[:], EPS, var[:], OP.add,
                                       OP.subtract)
        lnv = sm_p.tile([1, 512], FP, name="lnv", tag="lnv")
        nc.scalar.activation(lnv[:], var[:], AF.Ln)
        rstd = sm_p.tile([1, 512], FR, name="rstd", tag="rstd")
        nc.scalar.activation(rstd[:], lnv[:], AF.Exp, scale=-0.5)
        nmr = sm_p.tile([1, 512], FR, name="nmr", tag="nmr")
        nc.vector.scalar_tensor_tensor(nmr[:], mu[:], -1.0, rstd[:],
                                       OP.mult, OP.mult)
        ones1 = consts["ones1"]
        if PROJ_STEPS < 4:
            continue
        for fo in range(FO):
            pa = ps_a.tile([128, 512], FP, name="pa", tag="pa")
            nc.tensor.matmul(pa[:], gb[0:1, fo * 128:(fo + 1) * 128], rstd[:],
                             start=True, stop=True)
            # C = g * (-mu*rstd) + b  (b row lives at partition 32 of gb)
            pc = ps_c.tile([128, 512], FP, name="pc", tag="pc")
            nc.tensor.matmul(pc[:], gb[0:1, fo * 128:(fo + 1) * 128], nmr[:],
                             start=True, stop=False)
            nc.tensor.matmul(pc[:], b_row[0:1, fo * 128:(fo + 1) * 128],
                             ones1[:], start=False, stop=True)
            t = stg_p.tile([128, 512], FP, name="t", tag="t")
            nc.vector.tensor_mul(t[:], xs_sb[:, fo, :], pa[:])
            qn = stg_p.tile([128, 512], FR, name="qn", tag="qn")
            nc.vector.tensor_add(qn[:], t[:], pc[:])
            nc.sync.dma_start(dst_d[fo, :, rc * 512:(rc + 1) * 512], qn[:])


def _build(sq=1024, sk=2048, phases="qkvbc", reps=1):
    """sq: query rows per core; sk: key/value rows per core."""
    nc = bacc.Bacc("TRN2", target_bir_lowering=False, debug=False,
                   num_devices=N_CORES)
    xqT = nc.dram_tensor("xqT", [128, FI, sq], FR, kind="ExternalInput").ap()
    xkT = nc.dram_tensor("xkT", [128, FI, sk], FR, kind="ExternalInput").ap()
    xvT = nc.dram_tensor("xvT", [128, FI, sk], FR, kind="ExternalInput").ap()
    wq = nc.dram_tensor("wq", [128, FI, FO, 128], FR, kind="ExternalInput").ap()
    wk = nc.dram_tensor("wk", [128, FI, FO, 128], FR, kind="ExternalInput").ap()
    wv = nc.dram_tensor("wv", [128, FI, D], FR, kind="ExternalInput").ap()
    wo = nc.dram_tensor("wo", [128, HP, D], FR, kind="ExternalInput").ap()
    gbq_d = nc.dram_tensor("gbq", [33, D], FR, kind="ExternalInput").ap()
    gbk_d = nc.dram_tensor("gbk", [33, D], FR, kind="ExternalInput").ap()
    gv_d = nc.dram_tensor("gv", [128, D], FP, kind="ExternalInput").ap()
    bv_d = nc.dram_tensor("bv", [128, D], FP, kind="ExternalInput").ap()
    bo_d = nc.dram_tensor("bo", [128, D], FP, kind="ExternalInput").ap()
    out_d = nc.dram_tensor("out", [sq, D], FP, kind="ExternalOutput").ap()

    qt_d = nc.dram_tensor("qt_d", [FO, 128, sq], FR, kind="Internal").ap()
    kt_d = nc.dram_tensor("kt_d", [FO, 128, sk], FR, kind="Internal").ap()

    KB = sk // 128     # key blocks per head
    QC = sq // 512     # query chunks
    RBV = sk // 128    # V row blocks

    with tile.TileContext(nc) as tc:
        with tc.tile_pool(name="const", bufs=1) as cp, \
             tc.tile_pool(name="ohn", bufs=1) as ohn_p:
            ones_f = cp.tile([128, 512], FP, name="ones_f")
            nc.gpsimd.memset(ones_f[:], 1.0)
            ones_col = cp.tile([128, 1], FR, name="ones_col")
            nc.vector.tensor_copy(ones_col[:], ones_f[:, 0:1])
            ones1 = cp.tile([1, 512], FR, name="ones1")
            nc.vector.tensor_copy(ones1[:], ones_f[0:1, :])
            ones_bc = cp.tile([128, 64], FP, name="ones_bc")
            nc.gpsimd.memset(ones_bc[:], 1.0)
            gbq = cp.tile([33, D], FR, name="gbq_sb")
            nc.sync.dma_start(gbq[:], gbq_d[:])
            gbk = cp.tile([33, D], FR, name="gbk_sb")
            nc.sync.dma_start(gbk[:], gbk_d[:])
            bq_row = cp.tile([1, D], FR, name="bq_row")
            nc.vector.tensor_copy(bq_row[:], gbq[32:33, :])
            bk_row = cp.tile([1, D], FR, name="bk_row")
            nc.vector.tensor_copy(bk_row[:], gbk[32:33, :])
            gv_rep = cp.tile([128, D], FP, name="gv_rep")
            nc.sync.dma_start(gv_rep[:], gv_d[:])
            bv_rep = cp.tile([128, D], FP, name="bv_rep")
            nc.sync.dma_start(bv_rep[:], bv_d[:])
            bo_rep = cp.tile([128, D], FP, name="bo_rep")
            nc.sync.dma_start(bo_rep[:], bo_d[:])
            # OhT accumulator: [128 (pair-stacked dh), pair, q-rows]
            ohn = ohn_p.tile([128, HP, sq], FR, name="ohn_t")

            for _rep in range(reps):

              # ---------------- Phase A: Q and K projections (transposed) ----
              with tc.tile_pool(name="wqk", bufs=1) as w_p, \
                   tc.tile_pool(name="xa", bufs=2) as xa_p, \
                   tc.tile_pool(name="x2", bufs=3) as x2_p, \
                   tc.tile_pool(name="stg", bufs=4) as stg_p, \
                   tc.tile_pool(name="sm", bufs=2) as sm_p, \
                   tc.tile_pool(name="psp", bufs=2, space="PSUM") as ps_proj, \
                   tc.tile_pool(name="pss", bufs=2, space="PSUM") as ps_stats, \
                   tc.tile_pool(name="psa", bufs=2, space="PSUM") as ps_a, \
                   tc.tile_pool(name="psc", bufs=2, space="PSUM") as ps_c:
                  pools = (xa_p, x2_p, stg_p, sm_p, ps_proj, ps_stats, ps_a,
                           ps_c, {"ones_col": ones_col, "ones1": ones1})
                  if "q" in phases:
                    with nc.named_scope("proj_q"):
                      wq_sb = w_p.tile([128, FI, FO, 128], FR, name="wq_sb",
                                       tag="w")
                      nc.sync.dma_start(wq_sb[:], wq[:])
                      _proj_ln_transposed(nc, pools, xqT, wq_sb, gbq, bq_row, qt_d, sq)
                  if "k" in phases:
                    with nc.named_scope("proj_k"):
                      wk_sb = w_p.tile([128, FI, FO, 128], FR, name="wk_sb",
                                       tag="w")
                      nc.sync.dma_start(wk_sb[:], wk[:])
                      _proj_ln_transposed(nc, pools, xkT, wk_sb, gbk, bk_row, kt_d, sk)

              # denom/recip allocated after phase-A pools are released
              with tc.tile_pool(name="post", bufs=1) as post_p:
                # head h denominators at partition (h % 4)*32, free (h//4)*sq
                denom = post_p.tile([128, 4 * sq], FP, name="denom")
                nc.gpsimd.memset(denom[:], 1.0)

                # ------------- Phase A-V: V projection (natural) + V' -------
                with tc.tile_pool(name="vpool", bufs=1) as vp_p:
                    vprime = vp_p.tile([128, RBV, H * 65], FR, name="vprime")
                    for _rb in range(RBV):
                        nc.vector.tensor_copy(
                            vprime[:, _rb, :].rearrange(
                                "p (h c) -> p h c", c=65)[:, :, 64:65],
                            ones_f[:, 0:H])
                    if "v" in phases:
                      with nc.named_scope("proj_v"), \
                         tc.tile_pool(name="wv", bufs=1) as wv_p, \
                         tc.tile_pool(name="xv", bufs=2) as xv_p, \
                         tc.tile_pool(name="vst", bufs=2) as vst_p, \
                         tc.tile_pool(name="vsm", bufs=3) as vsm_p, \
                         tc.tile_pool(name="psv", bufs=2, space="PSUM") as ps_v:
                        wv_sb = wv_p.tile([128, FI, D], FR, name="wv_sb")
                        nc.sync.dma_start(wv_sb[:], wv[:])
                        for rb in range(RBV):
                            xv_sb = xv_p.tile([128, FI, 128], FR, name="xv_sb",
                                              tag="xv")
                            nc.sync.dma_start(
                                xv_sb[:], xvT[:, :, rb * 128:(rb + 1) * 128])
                            pv = ps_v.tile([128, D], FP, name="pv", tag="pv")
                            vsum = vsm_p.tile([128, 1], FP, name="vsum", tag="vs0")
                            vsum1 = vsm_p.tile([128, 1], FP, name="vsum1",
                                               tag="vs1")
                            v_sb = vst_p.tile([128, D], FP, name="v_sb", tag="v")
                            for half in range(2):
                                for fi in range(FI):
                                    nc.tensor.matmul(
                                        pv[:, half * 512:(half + 1) * 512],
                                        xv_sb[:, fi, :],
                                        wv_sb[:, fi, half * 512:(half + 1) * 512],
                                        start=(fi == 0), stop=(fi == FI - 1))
                                nc.vector.tensor_scalar(
                                    v_sb[:, half * 512:(half + 1) * 512],
                                    pv[:, half * 512:(half + 1) * 512],
                                    1.0, 0.0, OP.mult, OP.add,
                                    accum_out=(vsum[:] if half == 0 else vsum1[:]))
                            nc.vector.tensor_tensor(vsum[:], vsum[:], vsum1[:],
                                                    OP.add)
                            v2 = vst_p.tile([128, D], FP, name="v2", tag="v2",
                                            bufs=1)
                            vsq = vsm_p.tile([128, 1], FP, name="vsq", tag="vsq")
                            nc.vector.scalar_tensor_tensor(
                                v2[:], v_sb[:], 1.0, v_sb[:], OP.bypass, OP.mult,
                                accum_out=vsq[:])
                            mu_v = vsm_p.tile([128, 1], FP, name="mu_v", tag="muv")
                            nc.vector.tensor_scalar(mu_v[:], vsum[:], 1.0 / D,
                                                    None, OP.mult)
                            var_v = vsm_p.tile([128, 1], FP, name="var_v",
                                               tag="varv")
                            nc.vector.tensor_scalar(var_v[:], vsq[:], 1.0 / D,
                                                    None, OP.mult)
                            musq = vsm_p.tile([128, 1], FP, name="musq",
                                              tag="musq")
                            nc.vector.tensor_mul(musq[:], mu_v[:], mu_v[:])
                            nc.vector.scalar_tensor_tensor(
                                var_v[:], var_v[:], EPS, musq[:], OP.add,
                                OP.subtract)
                            lnv_v = vsm_p.tile([128, 1], FP, name="lnv_v",
                                               tag="lnvv")
                            nc.scalar.activation(lnv_v[:], var_v[:], AF.Ln)
                            rstd_v = vsm_p.tile([128, 1], FP, name="rstd_v",
                                                tag="rstdv")
                            nc.scalar.activation(rstd_v[:], lnv_v[:], AF.Exp,
                                                 scale=-0.5)
                            nmr_v = vsm_p.tile([128, 1], FP, name="nmr_v",
                                               tag="nmrv")
                            nc.vector.scalar_tensor_tensor(
                                nmr_v[:], mu_v[:], -1.0, rstd_v[:], OP.mult,
                                OP.mult)
                            nc.vector.tensor_scalar(v_sb[:], v_sb[:], rstd_v[:],
                                                    nmr_v[:], OP.mult, OP.add)
                            nc.vector.tensor_mul(v_sb[:], v_sb[:], gv_rep[:])
                            vp_dst = vprime[:, rb, :].rearrange(
                                "p (h c) -> p h c", c=65)[:, :, 0:64]
                            nc.vector.tensor_tensor(
                                vp_dst,
                                v_sb[:].rearrange("p (h c) -> p h c", c=64),
                                bv_rep[:].rearrange("p (h c) -> p h c", c=64),
                                OP.add)

                    # ---------------- Phase B: attention ----------------------
                    if "b" in phases:
                      with nc.named_scope("attn"), \
                         tc.tile_pool(name="kt", bufs=3) as kt_p, \
                         tc.tile_pool(name="qt", bufs=3) as qt_p, \
                         tc.tile_pool(name="exp", bufs=4) as exp_p, \
                         tc.tile_pool(name="psm", bufs=2, space="PSUM") as ps_sim, \
                         tc.tile_pool(name="pvA", bufs=2, space="PSUM") as ps_pva, \
                         tc.tile_pool(name="pvB", bufs=2, space="PSUM") as ps_pvb:
                        for hp in range(HP):
                            kt_sb = kt_p.tile([128, sk], FR, name="kt_sb",
                                              tag="kt")
                            nc.sync.dma_start(kt_sb[:], kt_d[hp, :, :])
                            qt_sb = qt_p.tile([128, sq], FR, name="qt_sb",
                                              tag="qt")
                            nc.sync.dma_start(qt_sb[:], qt_d[hp, :, :])
                            for qc in range(QC):
                                pva = ps_pva.tile([65, 512], FP, name="pva",
                                                  tag="pva")
                                pvb = ps_pvb.tile([65, 512], FP, name="pvb",
                                                  tag="pvb")
                                for kb in range(KB):
                                    sim = ps_sim.tile([128, 1024], FP, name="sim",
                                                      tag="sim")
                                    nc.tensor.matmul(
                                        sim[:, 0:512],
                                        kt_sb[0:64, kb * 128:(kb + 1) * 128],
                                        qt_sb[0:64, qc * 512:(qc + 1) * 512],
                                        start=True, stop=True)
                                    nc.tensor.matmul(
                                        sim[:, 512:1024],
                                        kt_sb[64:128, kb * 128:(kb + 1) * 128],
                                        qt_sb[64:128, qc * 512:(qc + 1) * 512],
                                        start=True, stop=True)
                                    ex = exp_p.tile([128, 1024], FR, name="ex",
                                                    tag="ex")
                                    nc.scalar.activation(ex[:], sim[:], AF.Exp)
                                    nc.tensor.matmul(
                                        pva[:],
                                        vprime[:, kb, (2 * hp) * 65:
                                               (2 * hp) * 65 + 65],
                                        ex[:, 0:512],
                                        start=(kb == 0), stop=(kb == KB - 1))
                                    nc.tensor.matmul(
                                        pvb[:],
                                        vprime[:, kb, (2 * hp + 1) * 65:
                                               (2 * hp + 1) * 65 + 65],
                                        ex[:, 512:1024],
                                        start=(kb == 0), stop=(kb == KB - 1))
                                qs = slice(qc * 512, (qc + 1) * 512)
                                nc.vector.tensor_copy(ohn[0:64, hp, qs],
                                                      pva[0:64, :])
                                nc.vector.tensor_copy(ohn[64:128, hp, qs],
                                                      pvb[0:64, :])
                                for hh, pv_ in ((2 * hp, pva), (2 * hp + 1, pvb)):
                                    pbase = (hh % 4) * 32
                                    foff = (hh // 4) * sq + qc * 512
                                    nc.vector.tensor_copy(
                                        denom[pbase:pbase + 1,
                                              foff:foff + 512], pv_[64:65, :])

                # ---------------- Phase C: normalize + output projection -------
                if "c" in phases:
                  with nc.named_scope("out_proj"), \
                     tc.tile_pool(name="wo", bufs=1) as wo_p, \
                     tc.tile_pool(name="ost", bufs=4) as ost_p, \
                     tc.tile_pool(name="psb", bufs=2, space="PSUM") as ps_bc, \
                     tc.tile_pool(name="pso", bufs=3, space="PSUM") as ps_o:
                    nc.vector.reciprocal(denom[:], denom[:])
                    wo_sb = wo_p.tile([128, HP, D], FR, name="wo_sb")
                    nc.sync.dma_start(wo_sb[:], wo[:])
                    for hp in range(HP):
                        bc = ps_bc.tile([128, sq], FP, name="bc", tag="bc")
                        for qc in range(QC):
                            qs = slice(qc * 512, (qc + 1) * 512)
                            for hh, obase in ((2 * hp, 0), (2 * hp + 1, 64)):
                                pbase = (hh % 4) * 32
                                foff = (hh // 4) * sq + qc * 512
                                nc.tensor.matmul(
                                    bc[obase:obase + 64, qs],
                                    ones_bc[pbase:pbase + 1, :],
                                    denom[pbase:pbase + 1, foff:foff + 512],
                                    start=True, stop=True,
                                    tile_position=(pbase, obase))
                        nc.vector.tensor_mul(ohn[:, hp, :], ohn[:, hp, :], bc[:])
                    for rb in range(sq // 128):
                        for half in range(2):
                            pso = ps_o.tile([128, 512], FP, name="pso", tag="pso")
                            for hp in range(HP):
                                nc.tensor.matmul(
                                    pso[:],
                                    ohn[:, hp, rb * 128:(rb + 1) * 128],
                                    wo_sb[:, hp, half * 512:(half + 1) * 512],
                                    start=(hp == 0), stop=(hp == HP - 1))
                            osb = ost_p.tile([128, 512], FP, name="osb",
                                             tag="osb")
                            nc.vector.tensor_add(
                                osb[:], pso[:],
                                bo_rep[:, half * 512:(half + 1) * 512])
                            nc.sync.dma_start(
                                out_d[rb * 128:(rb + 1) * 128,
                                      half * 512:(half + 1) * 512], osb[:])

    # All our ACT functions (Exp, Ln) live in natural_log_exp_and_others;
    # the greedy table chooser otherwise thrashes between the exp-only and
    # ln-only sets (~44 table loads x 1.3us on the ACT critical path).
    from concourse import bacc as _bacc_mod
    from concourse import mybir as _mb
    _orig_gat = _bacc_mod.get_activation_tables
    def _only_combined(arch):
        # Preserve dict order/size (act_func_set_id is positional); just
        # make the exp-only / ln-only sets unusable so the chooser lands
        # on the combined set for both functions.
        tabs = _orig_gat(arch)
        need = {_mb.ActivationFunctionType.Exp, _mb.ActivationFunctionType.Ln}
        out = {}
        for k, v in tabs.items():
            if (v & need) and not (need <= v):
                out[k] = set()
            else:
                out[k] = v
        return out
    _bacc_mod.get_activation_tables = _only_combined
    try:
        nc.compile()
    finally:
        _bacc_mod.get_activation_tables = _orig_gat
    return nc


_BUILT = {}
LAST_RESULTS = None


def _get_built(sq=1024, sk=2048, phases="qkvbc", reps=1):
    key = (sq, sk, phases, reps)
    if key not in _BUILT:
        _BUILT[key] = _build(sq, sk, phases, reps)
    return _BUILT[key]


def _tile_xt(x):
    """[rows, D] -> transposed tiled [128, FI, rows]."""
    return np.ascontiguousarray(
        x.T.reshape(FI, 128, x.shape[0]).transpose(1, 0, 2))


def _tile_w_blocks(w):
    """[D, D] -> [128, FI, FO, 128] where [p, fi, fo, :] = w[fi*128+p, fo*128:...]"""
    return np.ascontiguousarray(
        w.reshape(FI, 128, FO, 128).transpose(1, 0, 2, 3))


def _pack_gb(g, b):
    """gain at partition 0, bias at partition 32 (engine ops need 32-aligned
    partition bases)."""
    gb = np.zeros((33, D), np.float32)
    gb[0] = g
    gb[32] = b
    return gb


def _tile_w_rows(w, groups):
    """[D, D] -> [128, groups, D] where [p, g, :] = w[g*128+p, :]"""
    return np.ascontiguousarray(
        w.reshape(groups, 128, D).transpose(1, 0, 2))


def prepare_in_maps(query, key, value, Wq, gq, bq, Wk, gk, bk, Wv, gv, bv,
                    Wo, bo):
    f32 = lambda a: np.ascontiguousarray(np.asarray(a), dtype=np.float32)
    query, key, value = f32(query), f32(key), f32(value)
    Wq, Wk, Wv, Wo = f32(Wq), f32(Wk), f32(Wv), f32(Wo)
    gq, bq, gk, bk, gv, bv, bo = map(f32, (gq, bq, gk, bk, gv, bv, bo))
    scale = 1.0 / np.sqrt(np.float32(DH))
    common = {
        "wq": _tile_w_blocks(Wq),
        "wk": _tile_w_blocks(Wk),
        "wv": _tile_w_rows(Wv, FI),
        "wo": _tile_w_rows(Wo, HP),
        "gbq": _pack_gb(gq * scale, bq * scale),
        "gbk": _pack_gb(gk, bk),
        "gv": np.ascontiguousarray(np.broadcast_to(gv, (128, D))),
        "bv": np.ascontiguousarray(np.broadcast_to(bv, (128, D))),
        "bo": np.ascontiguousarray(np.broadcast_to(bo, (128, D))),
    }
    in_maps = []
    for c in range(N_CORES):
        b, half = divmod(c, 2)
        sl = slice(half * (S // 2), (half + 1) * (S // 2))
        in_maps.append({
            "xqT": _tile_xt(query[b, sl, :]),
            "xkT": _tile_xt(key[b]),
            "xvT": _tile_xt(value[b]),
            **common,
        })
    return in_maps


def assemble_out(results):
    out = np.empty((B, S, D), dtype=np.float32)
    for c in range(N_CORES):
        b, half = divmod(c, 2)
        sl = slice(half * (S // 2), (half + 1) * (S // 2))
        out[b, sl, :] = results[c]["out"]
    return out


def kernel(query, key, value, mask, Wq, gq, bq, Wk, gk, bk, Wv, gv, bv, Wo,
           bo):
    # mask is all-True in this problem; softmax runs over all keys.
    global LAST_RESULTS
    from concourse.bass_utils import run_bass_kernel_spmd

    nc = _get_built(S // 2, S)
    in_maps = prepare_in_maps(query, key, value, Wq, gq, bq, Wk, gk, bk,
                              Wv, gv, bv, Wo, bo)
    res = run_bass_kernel_spmd(nc, in_maps, core_ids=list(range(N_CORES)))
    LAST_RESULTS = res
    return assemble_out(res.results)

